# revision 2
# baseline (speedup 1.0000x reference)
"""Causal self-attention (B=4, S=2048, D=2048, H=16) on 8 Trainium2 cores.

Sharding: core c -> (batch b = c//2, head-half = c%2, i.e. 8 of 16 heads).
Megatron-style: Wq/Wk/Wv column-parallel, Wo row-parallel; each core emits a
partial (S, D) output for its batch; host sums the two half partials + bo.

All matmul operands fp16 (full-rate 1 cycle/row on the PE, f32 PSUM).
Fully fused single-pass program: x / V / ctx stay SBUF-resident (no DRAM
spills). Softmax denominator: DVE fp16 partial sums over k-chunks, one
ones[128,128] matmul fusing partition-reduce + broadcast, DVE reciprocal,
DVE multiply on the PSUM context.

The PE stream of phase A is issue-woven at matmul granularity: score
matmuls of head h interleave with AV matmuls (2 q-chunk units behind) and
next head's QKV chain blocks, so the Activation engine's exp throughput
(~766ns/tile vs 216ns/tile matmul) never stalls the PE on score-PSUM
reuse. The causal triangle is trimmed at 128 granularity in scores, exp,
AV, and the denominator adds. Phase B (out-projection) weaves the last
head's AV tail into its first tiles; prologue DMAs are spread over four
engine queues.

Device program per core:
  Phase 0: V = x @ Wv + bv for all 8 heads -> v_sb [s=128, 16 kc, 1024] fp16.
  Phase A per head h: K^T,Q^T chains (bias via Act) -> k_sb/q_sb [dk, 2048];
    per 512-q-chunk: S^T tiles [k,q], exp (scale=1/sqrt(dk)) -> es fp16,
    causal fill via gpsimd affine_select, ctx^T accum in PSUM, normalize.
  Phase B: out[q,e] = sum_h ctx_h^T.T @ Wo_h per 512-wide e-chunk.
"""

import math
from collections import deque

import numpy as np

import concourse.bass as bass
import concourse.mybir as mybir
from concourse.bass_utils import run_bass_kernel_spmd
from concourse.tile import TileContext

B, S, D, H = 4, 2048, 2048, 16
DK = 128
NCORES = 8
HPC = H // 2          # 8 heads per core
MLOC = HPC * DK       # 1024 local head dims
NSC = S // 512        # 4 q-chunks
NKC = S // 128        # 16 k-chunks

F32 = mybir.dt.float32
F16 = mybir.dt.float16
AF = mybir.ActivationFunctionType
SCALE = 1.0 / math.sqrt(DK)


def split_excess_waits(nc, max_waits=1):
    """walrus in this container accepts at most one sem-wait per instruction;
    move excess waits onto wait-only EventSemaphore insts inserted before."""
    ctr = 0
    for f in nc.m.functions:
        for bb in f.blocks:
            new = []
            changed = False
            for inst in bb.instructions:
                si = inst.sync_info
                if si is not None and si.on_wait and len(si.on_wait) > max_waits:
                    changed = True
                    waits = list(si.on_wait)
                    for w in waits[:-max_waits]:
                        ctr += 1
                        ev = mybir.InstEventSemaphore(
                            name=f"waitsplit-{ctr}", ins=[], outs=[],
                            sync_info=mybir.SyncInfo(on_wait=[w], on_update=[]))
                        ev.engine = inst.engine
                        new.append(ev)
                    si.on_wait = waits[-max_waits:]
                new.append(inst)
            if changed:
                bb.instructions = new
    return ctr


def build_nc():
    nc = bass.Bass()
    xb = nc.declare_dram_parameter("xb", [128, 16, S], F16, isOutput=False)
    wqb = nc.declare_dram_parameter("wqb", [128, HPC, 16, DK], F16, isOutput=False)
    wkb = nc.declare_dram_parameter("wkb", [128, HPC, 16, DK], F16, isOutput=False)
    wvb = nc.declare_dram_parameter("wvb", [128, 16, MLOC], F16, isOutput=False)
    wob = nc.declare_dram_parameter("wob", [128, HPC, D], F16, isOutput=False)
    bqb = nc.declare_dram_parameter("bqb", [DK, HPC], F32, isOutput=False)
    bkb = nc.declare_dram_parameter("bkb", [DK, HPC], F32, isOutput=False)
    bvb = nc.declare_dram_parameter("bvb", [MLOC], F16, isOutput=False)
    out = nc.declare_dram_parameter("out", [S, D], F32, isOutput=True)

    with TileContext(nc) as tc:
        with tc.tile_pool(name="res", bufs=1) as res, \
             tc.tile_pool(name="mmp", bufs=2, space="PSUM") as mmp, \
             tc.tile_pool(name="spp", bufs=3, space="PSUM") as spp, \
             tc.tile_pool(name="cpp", bufs=3, space="PSUM") as cpp:
            # ---- resident tensors + constants; initial DMAs spread over 4
            # engine queues so the prologue isn't one serialized queue ----
            qeng = [nc.sync, nc.scalar, nc.gpsimd]
            x_sb = res.tile([128, 16, S], F16)
            v_sb = res.tile([128, NKC, MLOC], F16)
            ctx_sb = res.tile([128, HPC, NSC, 512], F16)
            bq_sb = res.tile([DK, HPC], F32)
            bk_sb = res.tile([DK, HPC], F32)
            ones_sq = res.tile([128, 128], F16)
            nc.vector.memset(ones_sq[:], 1.0)

            # ---------------- Phase 0: V projection (all heads) -------------
            with tc.tile_pool(name="p0w", bufs=1) as p0w:
                wv_sb = p0w.tile([128, 16, MLOC], F16)
                bv_sb = p0w.tile([128, MLOC], F16)
                # interleave first-needed pieces across queues
                nc.sync.dma_start(out=wv_sb[:, :, 0:512], in_=wvb[:, :, 0:512])
                nc.scalar.dma_start(out=x_sb[:, :, 0:256], in_=xb[:, :, 0:256])
                nc.gpsimd.dma_start(out=wv_sb[:, :, 512:1024],
                                    in_=wvb[:, :, 512:1024])
                nc.scalar.dma_start(out=x_sb[:, :, 256:512],
                                    in_=xb[:, :, 256:512])
                for xc in range(2, 8):
                    qeng[xc % 3].dma_start(out=x_sb[:, :, xc*256:(xc+1)*256],
                                           in_=xb[:, :, xc*256:(xc+1)*256])
                nc.sync.dma_start(out=bq_sb[:], in_=bqb[:])
                nc.sync.dma_start(out=bk_sb[:], in_=bkb[:])
                nc.sync.dma_start(out=bv_sb[:],
                                  in_=bvb[:].partition_broadcast(128))
                for sc in range(NKC):
                    for hf in range(2):
                        vp = mmp.tile([128, 512], F32, tag="mm", name="vp")
                        for dc in range(16):
                            nc.tensor.matmul(
                                vp[:], x_sb[:, dc, sc*128:(sc+1)*128],
                                wv_sb[:, dc, hf*512:(hf+1)*512],
                                start=(dc == 0), stop=(dc == 15))
                        nc.vector.tensor_add(
                            v_sb[:, sc, hf*512:(hf+1)*512], vp[:],
                            bv_sb[:, hf*512:(hf+1)*512])

            # ---------------- Phase A: woven QKV + attention ----------------
            with tc.tile_pool(name="esp", bufs=1) as esp, \
                 tc.tile_pool(name="smp", bufs=2) as smp:
                hstate = {}
                ustate = {}

                def scores_item(h, qc, kc):
                    k_sb, q_sb = hstate[h]
                    nk = 4 * qc + 4
                    if kc == 0:
                        ustate[(h, qc)] = esp.tile([128, nk, 512], F16,
                                                   tag=f"es{qc}", name="es")
                    es = ustate[(h, qc)]
                    j = kc - 4 * qc
                    lo = 128 * j if j > 0 else 0
                    sp = spp.tile([128, 512], F32, tag="sp", name="sp")
                    nc.tensor.matmul(
                        sp[:, lo:], k_sb[:, kc*128:(kc+1)*128],
                        q_sb[:, qc*512+lo:(qc+1)*512],
                        start=True, stop=True)
                    nc.scalar.activation(es[:, kc, lo:], sp[:, lo:],
                                         AF.Exp, bias=0.0, scale=SCALE)

                class AvUnit:
                    """AV matmuls + denominator adds for one (h, qc) unit,
                    then the normalize step. One step() call per k-chunk
                    plus one final normalize step."""

                    def __init__(self, h, qc):
                        self.h, self.qc = h, qc
                        self.nk = 4 * qc + 4
                        self.kc = 0
                        self.es = ustate.pop((h, qc))
                        self.acc = smp.tile([128, 512], F16, tag="acc",
                                            name="acc")
                        self.ctxp = cpp.tile([128, 512], F32, tag="cp",
                                             name="ctxp")

                    def done(self):
                        return self.kc > self.nk

                    def step(self):
                        h, qc, kc, es = self.h, self.qc, self.kc, self.es
                        if kc == self.nk:   # normalize
                            den_b = spp.tile([128, 512], F32, tag="sp",
                                             name="denb")
                            nc.tensor.matmul(den_b[:], ones_sq[:], self.acc[:],
                                             start=True, stop=True)
                            rcpb = smp.tile([128, 512], F32, tag="rcpb",
                                            name="rcpb")
                            nc.vector.reciprocal(rcpb[:], den_b[:])
                            nc.vector.tensor_mul(ctx_sb[:, h, qc, :],
                                                 self.ctxp[:], rcpb[:])
                            self.kc += 1
                            return
                        j = kc - 4 * qc
                        lo = 128 * j if j > 0 else 0
                        if j >= 0:   # diagonal: causal fill inside [lo:]
                            nc.gpsimd.affine_select(
                                out=es[:, kc, lo:], in_=es[:, kc, lo:],
                                compare_op=mybir.AluOpType.is_ge,
                                fill=0.0, base=0,
                                pattern=[[1, 512 - lo]], channel_multiplier=-1)
                        if kc == 0:
                            nc.vector.tensor_copy(self.acc[:], es[:, kc, :])
                        else:
                            nc.vector.tensor_add(self.acc[:, lo:],
                                                 self.acc[:, lo:],
                                                 es[:, kc, lo:])
                        nc.tensor.matmul(
                            self.ctxp[:, lo:], v_sb[:, kc, h*128:(h+1)*128],
                            es[:, kc, lo:], start=(kc == 0),
                            stop=(kc == self.nk - 1), skip_group_check=True)
                        self.kc += 1

                pend = deque()
                avst = {"cur": None}

                def step_A():
                    cur = avst["cur"]
                    if cur is None:
                        if len(pend) > 2:
                            h2, qc2 = pend.popleft()
                            cur = avst["cur"] = AvUnit(h2, qc2)
                        else:
                            return
                    cur.step()
                    if cur.done():
                        avst["cur"] = None

                def step_A_forced():
                    cur = avst["cur"]
                    if cur is None:
                        if not pend:
                            return False
                        h2, qc2 = pend.popleft()
                        cur = avst["cur"] = AvUnit(h2, qc2)
                    cur.step()
                    if cur.done():
                        avst["cur"] = None
                    return True

                with tc.tile_pool(name="wqk", bufs=2) as wqk, \
                     tc.tile_pool(name="qkt", bufs=2) as qkt:
                    wq_t = {}
                    wk_t = {}

                    def load_w(h):
                        wq_t[h] = wqk.tile([128, 16, DK], F16, tag="wq",
                                           name="wqt")
                        nc.sync.dma_start(out=wq_t[h][:], in_=wqb[:, h])
                        wk_t[h] = wqk.tile([128, 16, DK], F16, tag="wk",
                                           name="wkt")
                        nc.sync.dma_start(out=wk_t[h][:], in_=wkb[:, h])

                    def chain_blocks(h):
                        """qkv(h) as a list of 4-matmul block callables; bias
                        act fires with each chain's last block."""
                        k_sb = qkt.tile([128, S], F16, tag="k", name="ksb")
                        q_sb = qkt.tile([128, S], F16, tag="q", name="qsb")
                        hstate[h] = (k_sb, q_sb)
                        blocks = []
                        for w_t, o_sb, b_sb in ((wk_t[h], k_sb, bk_sb),
                                                (wq_t[h], q_sb, bq_sb)):
                            for sc in range(NSC):
                                ps = {}
                                for bi in range(4):
                                    def blk(bi=bi, w_t=w_t, o_sb=o_sb,
                                            b_sb=b_sb, sc=sc, ps=ps, h=h):
                                        if bi == 0:
                                            ps["t"] = mmp.tile(
                                                [128, 512], F32, tag="mm",
                                                name="qkp")
                                        for dc in range(4*bi, 4*bi+4):
                                            nc.tensor.matmul(
                                                ps["t"][:], w_t[:, dc, :],
                                                x_sb[:, dc,
                                                     sc*512:(sc+1)*512],
                                                start=(dc == 0),
                                                stop=(dc == 15))
                                        if bi == 3:
                                            nc.scalar.activation(
                                                o_sb[:, sc*512:(sc+1)*512],
                                                ps["t"][:], AF.Identity,
                                                bias=b_sb[:, h:h+1],
                                                scale=1.0)
                                    blocks.append(blk)
                        return blocks

                    load_w(0)
                    load_w(1)
                    # head 0 QKV runs unwoven (nothing to overlap yet)
                    for blk in chain_blocks(0):
                        blk()
                    for h in range(HPC):
                        if h + 2 < HPC:
                            load_w(h + 2)
                        cblocks = chain_blocks(h + 1) if h + 1 < HPC else []
                        sitems = [(h, qc, kc) for qc in range(NSC)
                                  for kc in range(4 * qc + 4)]
                        si = ci = 0
                        while si < len(sitems) or ci < len(cblocks):
                            if si < len(sitems):
                                hh, qc, kc = sitems[si]
                                scores_item(hh, qc, kc)
                                si += 1
                                if kc == 4 * qc + 3:
                                    pend.append((hh, qc))
                            step_A()
                            if ci < len(cblocks):
                                cblocks[ci]()
                                ci += 1
                            step_A()
                        hstate.pop(h - 1, None)
                    # drain until only the last two units remain; those
                    # weave into phase B (their ctx is read late there)
                    while avst["cur"] is not None or len(pend) > 2 or (
                            pend and not (pend[0][0] == HPC - 1
                                          and pend[0][1] >= 2)):
                        if not step_A_forced():
                            break

                # ------------- Phase B: output projection -------------------
                with tc.tile_pool(name="wop", bufs=2) as wop, \
                     tc.tile_pool(name="op", bufs=2) as op:
                    for ec in range(4):
                        wo_t = wop.tile([128, HPC, 512], F16, tag="wo",
                                        name="wot")
                        nc.scalar.dma_start(out=wo_t[:],
                                            in_=wob[:, :, ec*512:(ec+1)*512])
                        for qc in range(NSC):
                            for qs in range(4):
                                for _ in range(3):
                                    step_A_forced()
                                ops = mmp.tile([128, 512], F32, tag="mm",
                                               name="ops")
                                for h2 in range(HPC):
                                    nc.tensor.matmul(
                                        ops[:],
                                        ctx_sb[:, h2, qc, qs*128:(qs+1)*128],
                                        wo_t[:, h2, :],
                                        start=(h2 == 0), stop=(h2 == HPC - 1))
                                o_sb = op.tile([128, 512], F32, tag="o",
                                               name="osb")
                                nc.scalar.activation(o_sb[:], ops[:], AF.Copy)
                                nc.sync.dma_start(
                                    out=out[qc*512+qs*128:qc*512+(qs+1)*128,
                                            ec*512:(ec+1)*512],
                                    in_=o_sb[:])
    split_excess_waits(nc)
    return nc


_NC_CACHE = {}


def _get_nc():
    if "nc" not in _NC_CACHE:
        _NC_CACHE["nc"] = build_nc()
    return _NC_CACHE["nc"]


def make_in_maps(x, Wq, bq, Wk, bk, Wv, bv, Wo, bo):
    f16 = np.float16
    f32 = np.float32
    in_maps = []
    for c in range(NCORES):
        b = c // 2
        half = c % 2
        sl = slice(half * MLOC, (half + 1) * MLOC)
        xT = np.ascontiguousarray(x[b].T)                       # [D, S]
        wqT = Wq[sl, :].T                                       # [D, MLOC]
        wkT = Wk[sl, :].T
        wvT = Wv[sl, :].T
        woT = Wo[:, sl].T                                       # [MLOC, D]
        in_maps.append({
            "xb": xT.reshape(16, 128, S).transpose(1, 0, 2).astype(f16),
            "wqb": wqT.reshape(16, 128, HPC, DK).transpose(1, 2, 0, 3).astype(f16),
            "wkb": wkT.reshape(16, 128, HPC, DK).transpose(1, 2, 0, 3).astype(f16),
            "wvb": wvT.reshape(16, 128, MLOC).transpose(1, 0, 2).astype(f16),
            "wob": woT.reshape(HPC, 128, D).transpose(1, 0, 2).astype(f16),
            "bqb": np.ascontiguousarray(bq[sl].reshape(HPC, DK).T, dtype=f32),
            "bkb": np.ascontiguousarray(bk[sl].reshape(HPC, DK).T, dtype=f32),
            "bvb": np.ascontiguousarray(bv[sl]).astype(f16),
        })
    return in_maps


def run(inputs, trace=False, trace_kwargs=None):
    x = np.asarray(inputs["x"], dtype=np.float32)
    nb, seq, d = x.shape
    nc = _get_nc()
    in_maps = make_in_maps(
        x, np.asarray(inputs["Wq"]), np.asarray(inputs["bq"]),
        np.asarray(inputs["Wk"]), np.asarray(inputs["bk"]),
        np.asarray(inputs["Wv"]), np.asarray(inputs["bv"]),
        np.asarray(inputs["Wo"]), np.asarray(inputs["bo"]))
    res = run_bass_kernel_spmd(nc, in_maps, list(range(NCORES)), trace=trace,
                               **(trace_kwargs or {}))
    bo = np.asarray(inputs["bo"], dtype=np.float32)
    out = np.empty((nb, seq, d), dtype=np.float32)
    for b in range(nb):
        out[b] = res.results[2*b]["out"] + res.results[2*b+1]["out"] + bo
    return out, res


def kernel(**inputs):
    out, _ = run(inputs, trace=False)
    return out


# revision 3
# speedup vs baseline: 1.1609x; 1.1609x over previous
"""Causal self-attention (B=4, S=2048, D=2048, H=16) on 8 Trainium2 cores.

Sharding: core c -> (batch b = c//2, head-half = c%2, i.e. 8 of 16 heads).
Megatron-style: Wq/Wk/Wv column-parallel, Wo row-parallel; each core emits a
partial (S, D) output for its batch; host sums the two half partials + bo.

All matmul operands fp16 (full-rate 1 cycle/row on the PE, f32 PSUM).
Fully fused single-pass program: x / V / ctx stay SBUF-resident (no DRAM
spills). Softmax denominator: DVE fp16 partial sums over k-chunks, one
ones[128,128] matmul fusing partition-reduce + broadcast, DVE reciprocal,
DVE multiply on the PSUM context.

The PE stream of phase A is issue-woven at matmul granularity: score
matmuls of head h interleave with AV matmuls (2 q-chunk units behind) and
next head's QKV chain blocks, so the Activation engine's exp throughput
(~766ns/tile vs 216ns/tile matmul) never stalls the PE on score-PSUM
reuse. The causal triangle is trimmed at 128 granularity in scores, exp,
AV, and the denominator adds. Phase B (out-projection) weaves the last
head's AV tail into its first tiles; prologue DMAs are spread over four
engine queues.

Device program per core:
  Phase 0: V = x @ Wv + bv for all 8 heads -> v_sb [s=128, 16 kc, 1024] fp16.
  Phase A per head h: K^T,Q^T chains (bias via Act) -> k_sb/q_sb [dk, 2048];
    per 512-q-chunk: S^T tiles [k,q], exp (scale=1/sqrt(dk)) -> es fp16,
    causal fill via gpsimd affine_select, ctx^T accum in PSUM, normalize.
  Phase B: out[q,e] = sum_h ctx_h^T.T @ Wo_h per 512-wide e-chunk.
"""

import math
from collections import deque

import numpy as np

import concourse.bass as bass
import concourse.mybir as mybir
from concourse.bass_utils import run_bass_kernel_spmd
from concourse.tile import TileContext

B, S, D, H = 4, 2048, 2048, 16
DK = 128
NCORES = 8
HPC = H // 2          # 8 heads per core
MLOC = HPC * DK       # 1024 local head dims
NSC = S // 512        # 4 q-chunks
NKC = S // 128        # 16 k-chunks

F32 = mybir.dt.float32
F16 = mybir.dt.float16
AF = mybir.ActivationFunctionType
SCALE = 1.0 / math.sqrt(DK)


def split_excess_waits(nc, max_waits=1):
    """walrus in this container accepts at most one sem-wait per instruction;
    move excess waits onto wait-only EventSemaphore insts inserted before."""
    ctr = 0
    for f in nc.m.functions:
        for bb in f.blocks:
            new = []
            changed = False
            for inst in bb.instructions:
                si = inst.sync_info
                if si is not None and si.on_wait and len(si.on_wait) > max_waits:
                    changed = True
                    waits = list(si.on_wait)
                    for w in waits[:-max_waits]:
                        ctr += 1
                        ev = mybir.InstEventSemaphore(
                            name=f"waitsplit-{ctr}", ins=[], outs=[],
                            sync_info=mybir.SyncInfo(on_wait=[w], on_update=[]))
                        ev.engine = inst.engine
                        new.append(ev)
                    si.on_wait = waits[-max_waits:]
                new.append(inst)
            if changed:
                bb.instructions = new
    return ctr


def build_nc():
    nc = bass.Bass()
    xb = nc.declare_dram_parameter("xb", [128, 16, S], F16, isOutput=False)
    wqb = nc.declare_dram_parameter("wqb", [128, HPC, 16, DK], F16, isOutput=False)
    wkb = nc.declare_dram_parameter("wkb", [128, HPC, 16, DK], F16, isOutput=False)
    wvb = nc.declare_dram_parameter("wvb", [128, 16, MLOC], F16, isOutput=False)
    wob = nc.declare_dram_parameter("wob", [128, HPC, D], F16, isOutput=False)
    bqb = nc.declare_dram_parameter("bqb", [DK, HPC], F32, isOutput=False)
    bkb = nc.declare_dram_parameter("bkb", [DK, HPC], F32, isOutput=False)
    bvb = nc.declare_dram_parameter("bvb", [MLOC], F16, isOutput=False)
    out = nc.declare_dram_parameter("out", [S, D], F32, isOutput=True)

    with TileContext(nc) as tc:
        with tc.tile_pool(name="res", bufs=1) as res, \
             tc.tile_pool(name="mmp", bufs=2, space="PSUM") as mmp, \
             tc.tile_pool(name="spp", bufs=3, space="PSUM") as spp, \
             tc.tile_pool(name="cpp", bufs=3, space="PSUM") as cpp:
            # ---- resident tensors + constants; initial DMAs spread over 4
            # engine queues so the prologue isn't one serialized queue ----
            qeng = [nc.sync, nc.scalar, nc.gpsimd]
            # x as four per-512-column tiles: one DMA each, so a reader
            # depends only on its own column range (interval dep-tracking
            # otherwise makes every read wait on all x DMAs)
            xts = [res.tile([128, 16, 512], F16, tag=f"x{i}", name="xt")
                   for i in range(4)]
            v_sb = res.tile([128, NKC, MLOC], F16)
            ctx_sb = res.tile([128, HPC, NSC, 512], F16)
            bq_sb = res.tile([DK, HPC], F32)
            bk_sb = res.tile([DK, HPC], F32)
            ones_sq = res.tile([128, 128], F16)
            nc.vector.memset(ones_sq[:], 1.0)

            # ---------------- Phase 0: V projection (all heads) -------------
            with tc.tile_pool(name="p0w", bufs=1) as p0w:
                wv_sb = p0w.tile([128, 16, MLOC], F16)
                bv_sb = p0w.tile([128, MLOC], F16)
                # waves: HBM path is ~450GB/s aggregate, so only the
                # first-needed 4MB (xts[0] + wv half 0) is in flight first;
                # later waves queue behind them. V loop is hf-outer so the
                # second wv half isn't needed for ~54us.
                nc.gpsimd.dma_start(out=xts[0][:], in_=xb[:, :, 0:512])
                nc.sync.dma_start(out=wv_sb[:, :, 0:512], in_=wvb[:, :, 0:512])
                nc.gpsimd.dma_start(out=xts[1][:], in_=xb[:, :, 512:1024])
                nc.sync.dma_start(out=xts[2][:], in_=xb[:, :, 1024:1536])
                nc.gpsimd.dma_start(out=xts[3][:], in_=xb[:, :, 1536:2048])
                nc.sync.dma_start(out=wv_sb[:, :, 512:1024],
                                  in_=wvb[:, :, 512:1024])
                nc.scalar.dma_start(out=bv_sb[:],
                                    in_=bvb[:].partition_broadcast(128))
                nc.scalar.dma_start(out=bq_sb[:], in_=bqb[:])
                nc.scalar.dma_start(out=bk_sb[:], in_=bkb[:])
                for hf in range(2):
                    for sc in range(NKC):
                        vp = mmp.tile([128, 512], F32, tag="mm", name="vp")
                        for dc in range(16):
                            nc.tensor.matmul(
                                vp[:],
                                xts[sc // 4][:, dc,
                                             (sc % 4)*128:(sc % 4 + 1)*128],
                                wv_sb[:, dc, hf*512:(hf+1)*512],
                                start=(dc == 0), stop=(dc == 15))
                        nc.vector.tensor_add(
                            v_sb[:, sc, hf*512:(hf+1)*512], vp[:],
                            bv_sb[:, hf*512:(hf+1)*512])

            # ---------------- Phase A: woven QKV + attention ----------------
            with tc.tile_pool(name="esp", bufs=1) as esp, \
                 tc.tile_pool(name="smp", bufs=2) as smp:
                hstate = {}
                ustate = {}

                def scores_item(h, qc, kc):
                    k_sb, q_sb = hstate[h]
                    nk = 4 * qc + 4
                    if kc == 0:
                        ustate[(h, qc)] = esp.tile([128, nk, 512], F16,
                                                   tag=f"es{qc}", name="es")
                    es = ustate[(h, qc)]
                    j = kc - 4 * qc
                    lo = 128 * j if j > 0 else 0
                    sp = spp.tile([128, 512], F32, tag="sp", name="sp")
                    nc.tensor.matmul(
                        sp[:, lo:], k_sb[:, kc*128:(kc+1)*128],
                        q_sb[:, qc*512+lo:(qc+1)*512],
                        start=True, stop=True)
                    nc.scalar.activation(es[:, kc, lo:], sp[:, lo:],
                                         AF.Exp, bias=0.0, scale=SCALE)

                class AvUnit:
                    """AV matmuls + denominator adds for one (h, qc) unit,
                    then the normalize step. One step() call per k-chunk
                    plus one final normalize step."""

                    def __init__(self, h, qc):
                        self.h, self.qc = h, qc
                        self.nk = 4 * qc + 4
                        self.kc = 0
                        self.es = ustate.pop((h, qc))
                        self.acc = smp.tile([128, 512], F16, tag="acc",
                                            name="acc")
                        self.ctxp = cpp.tile([128, 512], F32, tag="cp",
                                             name="ctxp")

                    def done(self):
                        return self.kc > self.nk

                    def step(self):
                        h, qc, kc, es = self.h, self.qc, self.kc, self.es
                        if kc == self.nk:   # normalize
                            den_b = spp.tile([128, 512], F32, tag="sp",
                                             name="denb")
                            nc.tensor.matmul(den_b[:], ones_sq[:], self.acc[:],
                                             start=True, stop=True)
                            rcpb = smp.tile([128, 512], F32, tag="rcpb",
                                            name="rcpb")
                            nc.vector.reciprocal(rcpb[:], den_b[:])
                            nc.vector.tensor_mul(ctx_sb[:, h, qc, :],
                                                 self.ctxp[:], rcpb[:])
                            self.kc += 1
                            return
                        j = kc - 4 * qc
                        lo = 128 * j if j > 0 else 0
                        if j >= 0:   # diagonal: causal fill inside [lo:]
                            nc.gpsimd.affine_select(
                                out=es[:, kc, lo:], in_=es[:, kc, lo:],
                                compare_op=mybir.AluOpType.is_ge,
                                fill=0.0, base=0,
                                pattern=[[1, 512 - lo]], channel_multiplier=-1)
                        if kc == 0:
                            nc.vector.tensor_copy(self.acc[:], es[:, kc, :])
                        else:
                            nc.vector.tensor_add(self.acc[:, lo:],
                                                 self.acc[:, lo:],
                                                 es[:, kc, lo:])
                        nc.tensor.matmul(
                            self.ctxp[:, lo:], v_sb[:, kc, h*128:(h+1)*128],
                            es[:, kc, lo:], start=(kc == 0),
                            stop=(kc == self.nk - 1), skip_group_check=True)
                        self.kc += 1

                pend = deque()
                avst = {"cur": None}

                def step_A(stagger=2):
                    cur = avst["cur"]
                    if cur is None:
                        if len(pend) > stagger:
                            h2, qc2 = pend.popleft()
                            cur = avst["cur"] = AvUnit(h2, qc2)
                        else:
                            return
                    cur.step()
                    if cur.done():
                        avst["cur"] = None

                def step_A_forced():
                    cur = avst["cur"]
                    if cur is None:
                        if not pend:
                            return False
                        h2, qc2 = pend.popleft()
                        cur = avst["cur"] = AvUnit(h2, qc2)
                    cur.step()
                    if cur.done():
                        avst["cur"] = None
                    return True

                with tc.tile_pool(name="wqk", bufs=2) as wqk, \
                     tc.tile_pool(name="qkt", bufs=2) as qkt:
                    wq_t = {}
                    wk_t = {}

                    def load_w(h):
                        wq_t[h] = wqk.tile([128, 16, DK], F16, tag="wq",
                                           name="wqt")
                        nc.sync.dma_start(out=wq_t[h][:], in_=wqb[:, h])
                        wk_t[h] = wqk.tile([128, 16, DK], F16, tag="wk",
                                           name="wkt")
                        nc.sync.dma_start(out=wk_t[h][:], in_=wkb[:, h])

                    def chain_blocks(h):
                        """qkv(h) as a list of 4-matmul block callables; bias
                        act fires with each chain's last block."""
                        k_sb = qkt.tile([128, S], F16, tag="k", name="ksb")
                        q_sb = qkt.tile([128, S], F16, tag="q", name="qsb")
                        hstate[h] = (k_sb, q_sb)
                        blocks = []
                        for sc in range(NSC):
                            for w_t, o_sb, b_sb in ((wk_t[h], k_sb, bk_sb),
                                                    (wq_t[h], q_sb, bq_sb)):
                                ps = {}
                                for bi in range(4):
                                    def blk(bi=bi, w_t=w_t, o_sb=o_sb,
                                            b_sb=b_sb, sc=sc, ps=ps, h=h):
                                        if bi == 0:
                                            ps["t"] = mmp.tile(
                                                [128, 512], F32, tag="mm",
                                                name="qkp")
                                        for dc in range(4*bi, 4*bi+4):
                                            nc.tensor.matmul(
                                                ps["t"][:], w_t[:, dc, :],
                                                xts[sc][:, dc, :],
                                                start=(dc == 0),
                                                stop=(dc == 15))
                                        if bi == 3:
                                            nc.scalar.activation(
                                                o_sb[:, sc*512:(sc+1)*512],
                                                ps["t"][:], AF.Identity,
                                                bias=b_sb[:, h:h+1],
                                                scale=1.0)
                                    blocks.append(blk)
                        return blocks

                    load_w(0)
                    load_w(1)
                    # head 0 QKV runs unwoven (nothing to overlap yet)
                    for blk in chain_blocks(0):
                        blk()
                    for h in range(HPC):
                        if h + 2 < HPC:
                            load_w(h + 2)
                        cblocks = chain_blocks(h + 1) if h + 1 < HPC else []
                        sitems = [(h, qc, kc) for qc in range(NSC)
                                  for kc in range(4 * qc + 4)]
                        si = ci = 0
                        while si < len(sitems) or ci < len(cblocks):
                            if si < len(sitems):
                                hh, qc, kc = sitems[si]
                                scores_item(hh, qc, kc)
                                si += 1
                                if kc == 4 * qc + 3:
                                    pend.append((hh, qc))
                            step_A()
                            if ci < len(cblocks):
                                cblocks[ci]()
                                ci += 1
                            step_A()
                        hstate.pop(h - 1, None)
                    # drain until only the last two units remain; those
                    # weave into phase B (their ctx is read late there)
                    while avst["cur"] is not None or len(pend) > 2 or (
                            pend and not (pend[0][0] == HPC - 1
                                          and pend[0][1] >= 2)):
                        if not step_A_forced():
                            break

                # ------------- Phase B: output projection -------------------
                with tc.tile_pool(name="wop", bufs=2) as wop, \
                     tc.tile_pool(name="op", bufs=2) as op:
                    for ec in range(4):
                        wo_t = wop.tile([128, HPC, 512], F16, tag="wo",
                                        name="wot")
                        nc.scalar.dma_start(out=wo_t[:],
                                            in_=wob[:, :, ec*512:(ec+1)*512])
                        for qc in range(NSC):
                            for qs in range(4):
                                for _ in range(3):
                                    step_A_forced()
                                ops = mmp.tile([128, 512], F32, tag="mm",
                                               name="ops")
                                for h2 in range(HPC):
                                    nc.tensor.matmul(
                                        ops[:],
                                        ctx_sb[:, h2, qc, qs*128:(qs+1)*128],
                                        wo_t[:, h2, :],
                                        start=(h2 == 0), stop=(h2 == HPC - 1))
                                o_sb = op.tile([128, 512], F32, tag="o",
                                               name="osb")
                                nc.scalar.activation(o_sb[:], ops[:], AF.Copy)
                                nc.sync.dma_start(
                                    out=out[qc*512+qs*128:qc*512+(qs+1)*128,
                                            ec*512:(ec+1)*512],
                                    in_=o_sb[:])
    split_excess_waits(nc)
    return nc


_NC_CACHE = {}


def _get_nc():
    if "nc" not in _NC_CACHE:
        _NC_CACHE["nc"] = build_nc()
    return _NC_CACHE["nc"]


def make_in_maps(x, Wq, bq, Wk, bk, Wv, bv, Wo, bo):
    f16 = np.float16
    f32 = np.float32
    in_maps = []
    for c in range(NCORES):
        b = c // 2
        half = c % 2
        sl = slice(half * MLOC, (half + 1) * MLOC)
        xT = np.ascontiguousarray(x[b].T)                       # [D, S]
        wqT = Wq[sl, :].T                                       # [D, MLOC]
        wkT = Wk[sl, :].T
        wvT = Wv[sl, :].T
        woT = Wo[:, sl].T                                       # [MLOC, D]
        in_maps.append({
            "xb": xT.reshape(16, 128, S).transpose(1, 0, 2).astype(f16),
            "wqb": wqT.reshape(16, 128, HPC, DK).transpose(1, 2, 0, 3).astype(f16),
            "wkb": wkT.reshape(16, 128, HPC, DK).transpose(1, 2, 0, 3).astype(f16),
            "wvb": wvT.reshape(16, 128, MLOC).transpose(1, 0, 2).astype(f16),
            "wob": woT.reshape(HPC, 128, D).transpose(1, 0, 2).astype(f16),
            "bqb": np.ascontiguousarray(bq[sl].reshape(HPC, DK).T, dtype=f32),
            "bkb": np.ascontiguousarray(bk[sl].reshape(HPC, DK).T, dtype=f32),
            "bvb": np.ascontiguousarray(bv[sl]).astype(f16),
        })
    return in_maps


def run(inputs, trace=False, trace_kwargs=None):
    x = np.asarray(inputs["x"], dtype=np.float32)
    nb, seq, d = x.shape
    nc = _get_nc()
    in_maps = make_in_maps(
        x, np.asarray(inputs["Wq"]), np.asarray(inputs["bq"]),
        np.asarray(inputs["Wk"]), np.asarray(inputs["bk"]),
        np.asarray(inputs["Wv"]), np.asarray(inputs["bv"]),
        np.asarray(inputs["Wo"]), np.asarray(inputs["bo"]))
    res = run_bass_kernel_spmd(nc, in_maps, list(range(NCORES)), trace=trace,
                               **(trace_kwargs or {}))
    bo = np.asarray(inputs["bo"], dtype=np.float32)
    out = np.empty((nb, seq, d), dtype=np.float32)
    for b in range(nb):
        out[b] = res.results[2*b]["out"] + res.results[2*b+1]["out"] + bo
    return out, res


def kernel(**inputs):
    out, _ = run(inputs, trace=False)
    return out
